# revision 8
# baseline (speedup 1.0000x reference)
"""DeepSeek-style MoE layer (8 experts, top-2, 1 shared expert) on 8 Trainium2
NeuronCores.

Strategy: host-routed expert parallelism.
  - The host computes the router exactly (f64 logits -> top-2 -> sigmoid
    gates), gathers each expert's tokens, and pads to a common capacity
    CAPMAX (multiple of 128).  Core e receives: its 1024-token shard of x
    (shared-expert data parallelism) plus the pre-gathered token batch of
    expert e with per-slot gates.
  - The device kernel is a pure dense GEMM pipeline (no router, no
    compaction, no indirect DMA): for each 512-token chunk, stream Wfc in
    2MB tiles -> fc matmuls (f32 PSUM) -> gelu (Act engine, bf16 hT), then
    proj with Wproj tiles held in SBUF across two 4-bank PSUM passes so
    4 banks are always free and chunk boundaries never stall the PE.
    Expert chunks apply the gate during the PSUM drain (DVE).
  - Weight traffic per core: one expert (16.8MB streamed per chunk) + the
    shared expert, ~120MB total vs ~170MB for full replication; PE runs
    dense bf16 matmuls at N=512 throughout.
  - Host combine: y[shard] = ys; y[idx_e] += ye_e[:cnt_e] (token indices
    are unique within one expert); proj biases added on host (exact).

kernel(**inputs) takes the full [4, 2048, 1024] f32 inputs and returns the
full [4, 2048, 1024] f32 output.
"""

import numpy as np
import ml_dtypes

# Model dims (hardcoded per harness contract)
B, T = 4, 2048
N = B * T             # 8192 tokens
C = 1024              # embed dim
H = 4096              # hidden dim
E = 8                 # routed experts
N_CORES = 8
TOKS = N // N_CORES   # shared-expert tokens per core
KC = C // 128         # contraction chunks over C
KH = H // 128         # contraction chunks over H

_NC_CACHE = {}


def _route(x, Wr):
    """Exact host-side routing: per-token top-2 experts and renormalized
    gates from f64 logits (margins are ~1e-5+, far above f32 noise, so this
    reproduces the f32 reference selection).  Returns (idxs, gates, capmax):
    idxs[e] = token ids routed to expert e (ascending), gates[e] = matching
    normalized top-2 weights, capmax = max count rounded up to 128."""
    xf = np.asarray(x, np.float64).reshape(N, C)
    logits = xf @ np.asarray(Wr, np.float64)
    order = np.argsort(-logits, axis=1, kind="stable")
    e1, e2 = order[:, 0], order[:, 1]
    ar = np.arange(N)
    g1 = 1.0 / (1.0 + np.exp(logits[ar, e2] - logits[ar, e1]))
    idxs, gates = [], []
    for e in range(E):
        m1 = e1 == e
        m2 = e2 == e
        idx = np.nonzero(m1 | m2)[0]
        g = np.where(m1[idx], g1[idx], 1.0 - g1[idx])
        idxs.append(idx)
        gates.append(g.astype(np.float32))
    capmax = max(len(i) for i in idxs)
    capmax = max(128, -(-capmax // 128) * 128)
    return idxs, gates, capmax


def _build_nc(capmax):
    import concourse.bacc as bacc
    import concourse.mybir as mybir
    import concourse.tile as tile

    dt = mybir.dt
    AF = mybir.ActivationFunctionType
    ALU = mybir.AluOpType
    F32, BF16 = dt.float32, dt.bfloat16
    # CoreSim doesn't implement Gelu_apprx_tanh; tests may override this to
    # "Tanh" to functionally validate the pipeline in simulation
    act_fn = getattr(AF, globals().get("_ACT_NAME", "Gelu_apprx_tanh"))

    nc = bacc.Bacc("TRN2", target_bir_lowering=False, debug=False,
                   num_devices=N_CORES)

    # ---- DRAM I/O ----
    xs = nc.dram_tensor("xs", [C, TOKS], BF16, kind="ExternalInput").ap()
    xe = nc.dram_tensor("xe", [C, capmax], BF16, kind="ExternalInput").ap()
    gt = nc.dram_tensor("gt", [128, capmax // 128], F32,
                        kind="ExternalInput").ap()
    wfcs = nc.dram_tensor("wfcs", [C, H], BF16, kind="ExternalInput").ap()
    wpjs = nc.dram_tensor("wpjs", [H, C], BF16, kind="ExternalInput").ap()
    wfce = nc.dram_tensor("wfce", [C, H], BF16, kind="ExternalInput").ap()
    wpje = nc.dram_tensor("wpje", [H, C], BF16, kind="ExternalInput").ap()
    # fc biases partition-major: [p, j] = b[j*128 + p]
    bfcs = nc.dram_tensor("bfcs", [128, KH], F32, kind="ExternalInput").ap()
    bfce = nc.dram_tensor("bfce", [128, KH], F32, kind="ExternalInput").ap()
    ys = nc.dram_tensor("ys", [TOKS, C], F32, kind="ExternalOutput").ap()
    ye = nc.dram_tensor("ye", [capmax, C], F32, kind="ExternalOutput").ap()

    # expert chunk sizes: 512s, remainder merged into the last chunk so no
    # chunk is weight-DMA-bound (a 128-token tail would be)
    ech = []
    r = capmax
    while r >= 512:
        ech.append(512)
        r -= 512
    if r:
        if ech:
            ech[-1] += r
        else:
            ech = [r]
    maxnt = max([512] + ech)

    with tile.TileContext(nc) as tc:
        with tc.tile_pool(name="xp", bufs=1) as xp, \
             tc.tile_pool(name="xh", bufs=3) as xh, \
             tc.tile_pool(name="wf", bufs=2) as wf, \
             tc.tile_pool(name="wp", bufs=4) as wp, \
             tc.tile_pool(name="hh", bufs=1) as hh, \
             tc.tile_pool(name="dr", bufs=2) as dr, \
             tc.tile_pool(name="ps", bufs=8, space="PSUM") as ps8:

            bfcs_sb = xp.tile([128, KH], F32)
            bfce_sb = xp.tile([128, KH], F32)
            gt_sb = xp.tile([128, capmax // 128], F32)
            xe_sb = xp.tile([128, KC, capmax], BF16)
            hT = hh.tile([128, KH, maxnt], BF16)

            # Everything rides one serial DMA bus; order matters.  Startup
            # chain: xs chunk 0 (1MB) + first half of Wfc_s (1MB) gate the
            # first matmul, so they lead the ring.
            xs_t = []
            for sch in range(2):
                xt = xh.tile([128, KC, 512], BF16, tag="xh",
                             name=f"xs_{sch}")
                if sch == 0:
                    nc.sync.dma_start(
                        out=xt[:],
                        in_=xs[:, sch * 512:(sch + 1) * 512]
                            .rearrange("(k p) t -> p k t", p=128))
                xs_t.append(xt)

            def chunk(cidx, xt_sb, t0, nt, wfc_ap, wpj_ap, bias_sb, out_ap,
                      ot0, gmt):
                """One nt-token chunk (nt % 128 == 0, nt <= 1024): stream Wfc
                in 2MB tiles -> fc+gelu into hT; stream Wproj into 4 tiles
                held across <=4-bank PSUM proj passes; gmt = gate col base."""
                parts = [(p0, min(512, nt - p0)) for p0 in range(0, nt, 512)]
                # ---- fc: hT[j, :nt] = gelu(Wfc[:, j]^T @ x + bfc) ----
                for kg in range(4):
                    if cidx == 0 and kg == 0:
                        # split first tile so the PE starts ~3us earlier
                        wta = xh.tile([128, KC, 512], BF16, tag="xh",
                                      name="wf0a")
                        wtb = xh.tile([128, KC, 512], BF16, tag="xh",
                                      name="wf0b")
                        for half, wt_ in ((0, wta), (1, wtb)):
                            nc.sync.dma_start(
                                out=wt_[:],
                                in_=wfc_ap[:, half * 512:(half + 1) * 512]
                                    .rearrange("(c p) h -> p c h", p=128))
                        nc.sync.dma_start(out=bfcs_sb[:], in_=bfcs)
                        nc.sync.dma_start(out=bfce_sb[:], in_=bfce)
                        nc.sync.dma_start(out=gt_sb[:], in_=gt)

                        def wsel(m):
                            return (wta if m < 4 else wtb), (m % 4) * 128
                    else:
                        wt = wf.tile([128, KC, 1024], BF16, tag="wf")
                        nc.sync.dma_start(
                            out=wt[:],
                            in_=wfc_ap[:, kg * 1024:(kg + 1) * 1024]
                                .rearrange("(c p) h -> p c h", p=128))

                        def wsel(m, wt=wt):
                            return wt, m * 128
                    for m in range(8):
                        j = kg * 8 + m
                        wt_, mo = wsel(m)
                        for pi, (p0, pn) in enumerate(parts):
                            ps = ps8.tile([128, pn], F32, tag="b",
                                          name=f"fc_{cidx}_{j}_{pi}")
                            for c in range(KC):
                                nc.tensor.matmul(
                                    ps[:],
                                    lhsT=wt_[:, c, mo:mo + 128],
                                    rhs=xt_sb[:, c, t0 + p0:t0 + p0 + pn],
                                    start=(c == 0), stop=(c == KC - 1))
                            nc.scalar.activation(hT[:, j, p0:p0 + pn], ps[:],
                                                 act_fn,
                                                 bias=bias_sb[:, j:j + 1],
                                                 scale=1.0)
                    if cidx == 0 and kg == 3:
                        # bus-idle window: expert tokens + 2nd shared x chunk
                        nc.sync.dma_start(
                            out=xe_sb[:],
                            in_=xe.rearrange("(k p) t -> p k t", p=128))
                        nc.sync.dma_start(
                            out=xs_t[1][:],
                            in_=xs[:, 512:1024]
                                .rearrange("(k p) t -> p k t", p=128))
                # ---- proj: y[t0:t0+nt] = hT^T @ Wproj (2-m-tile passes) ----
                nmt = nt // 128
                wts = [wp.tile([128, KC, 1024], BF16, tag="wp",
                               name=f"wp_{cidx}_{kg}")
                       for kg in range(4)]
                for kg in range(4):
                    nc.sync.dma_start(
                        out=wts[kg][:],
                        in_=wpj_ap[kg * 1024:(kg + 1) * 1024, :]
                            .rearrange("(k p) c -> p k c", p=128))
                for p0 in range(0, nmt, 2):
                    mts = range(p0, min(p0 + 2, nmt))
                    ps2 = {(m, ch): ps8.tile([128, 512], F32, tag="b",
                                             name=f"pj_{cidx}_{m}_{ch}")
                           for m in mts for ch in range(2)}
                    for kg in range(4):
                        for kk in range(8):
                            for m in mts:
                                for ch in range(2):
                                    nc.tensor.matmul(
                                        ps2[m, ch][:],
                                        lhsT=hT[:, kg * 8 + kk,
                                                m * 128:(m + 1) * 128],
                                        rhs=wts[kg][:, kk,
                                                    ch * 512:(ch + 1) * 512],
                                        start=(kg == 0 and kk == 0),
                                        stop=(kg == 3 and kk == 7))
                    for m in mts:
                        yo = dr.tile([128, C], F32, tag="yo")
                        for ch in range(2):
                            if gmt is None:
                                nc.vector.tensor_copy(
                                    yo[:, ch * 512:(ch + 1) * 512],
                                    ps2[m, ch][:])
                            else:
                                nc.vector.tensor_scalar(
                                    yo[:, ch * 512:(ch + 1) * 512],
                                    ps2[m, ch][:],
                                    gt_sb[:, gmt + m:gmt + m + 1], None,
                                    op0=ALU.mult)
                        # Act-queue DMA keeps the sync ring free for weights
                        nc.scalar.dma_start(
                            out=out_ap[ot0 + m * 128:ot0 + (m + 1) * 128, :],
                            in_=yo[:])

            cidx = 0
            for sc in range(2):
                chunk(cidx, xs_t[sc], 0, 512, wfcs, wpjs, bfcs_sb,
                      ys, sc * 512, None)
                cidx += 1
            off = 0
            for nt in ech:
                chunk(cidx, xe_sb, off, nt, wfce, wpje, bfce_sb,
                      ye, off, off // 128)
                cidx += 1
                off += nt

    nc.compile()
    return nc


def get_nc(capmax=None):
    if capmax is None:
        if _NC_CACHE:
            return next(iter(_NC_CACHE.values()))
        capmax = 2176
    if capmax not in _NC_CACHE:
        _NC_CACHE.clear()   # one compiled program at a time
        _NC_CACHE[capmax] = _build_nc(capmax)
    return _NC_CACHE[capmax]


def _prep_in_maps(x, Wfc_s, bfc_s, Wproj_s, bproj_s, Wr, Wfc, bfc, Wproj,
                  bproj):
    bf16 = ml_dtypes.bfloat16
    idxs, gates, capmax = _route(x, Wr)
    xf = np.asarray(x, np.float32).reshape(N, C)
    xT = np.ascontiguousarray(xf.T.astype(bf16))          # [C, N]
    wfcs_b = np.ascontiguousarray(np.asarray(Wfc_s, np.float32)).astype(bf16)
    wpjs_b = np.ascontiguousarray(np.asarray(Wproj_s, np.float32)).astype(bf16)
    wfc_b = np.ascontiguousarray(np.asarray(Wfc, np.float32)).astype(bf16)
    wpj_b = np.ascontiguousarray(np.asarray(Wproj, np.float32)).astype(bf16)
    bfcs_f = np.ascontiguousarray(
        np.asarray(bfc_s, np.float32).reshape(KH, 128).T)
    bfc_f = np.asarray(bfc, np.float32)
    in_maps = []
    for c in range(N_CORES):
        idx = idxs[c]
        cnt = len(idx)
        xeT = np.zeros((C, capmax), bf16)
        xeT[:, :cnt] = xT[:, idx]
        gtv = np.zeros(capmax, np.float32)
        gtv[:cnt] = gates[c]
        in_maps.append({
            "xs": np.ascontiguousarray(xT[:, c * TOKS:(c + 1) * TOKS]),
            "xe": xeT,
            "gt": np.ascontiguousarray(gtv.reshape(capmax // 128, 128).T),
            "wfcs": wfcs_b,
            "wpjs": wpjs_b,
            "wfce": np.ascontiguousarray(wfc_b[c]),
            "wpje": np.ascontiguousarray(wpj_b[c]),
            "bfcs": bfcs_f,
            "bfce": np.ascontiguousarray(bfc_f[c].reshape(KH, 128).T),
        })
    return in_maps


def kernel(x, Wfc_s, bfc_s, Wproj_s, bproj_s, Wr, Wfc, bfc, Wproj, bproj):
    from concourse.bass_utils import run_bass_kernel_spmd

    idxs, gates, capmax = _route(x, Wr)
    nc = get_nc(capmax)
    in_maps = _prep_in_maps(x, Wfc_s, bfc_s, Wproj_s, bproj_s, Wr, Wfc, bfc,
                            Wproj, bproj)
    res = run_bass_kernel_spmd(nc, in_maps, core_ids=list(range(N_CORES)))
    out = np.empty((N, C), np.float32)
    for c in range(N_CORES):
        out[c * TOKS:(c + 1) * TOKS] = res.results[c]["ys"]
    # routed partials are pre-gated on device; token ids unique per expert
    for c in range(N_CORES):
        idx = idxs[c]
        out[idx] += res.results[c]["ye"][:len(idx)]
    # proj biases applied on host (exact; zero in this problem)
    bps = np.asarray(bproj_s, np.float32)
    if bps.any():
        out += bps
    bpj = np.asarray(bproj, np.float32)
    if bpj.any():
        for c in range(E):
            out[idxs[c]] += gates[c][:, None] * bpj[c][None, :]
    return out.reshape(B, T, C)
